# revision 17
# baseline (speedup 1.0000x reference)
"""Bass/Tile kernel for a single causal attention head on 8 trn2 NeuronCores.

Problem: input [8, 2048, 768], Wq/Wk/Wv [768, 64] ->
  O = softmax(causal(Q K^T)/sqrt(64)) V, per batch.  [8, 2048, 64]

Sharding: data-parallel over batch; core b handles batch b. Weights
replicated.  No collectives needed.

All-bf16 matmul pipeline (tolerance 2e-2; measured rel err ~5e-3):
  - bf16 matmuls run 1 cyc/row incl. <256-wide outputs (fp32r degrades to
    4 there), bf16 PE transposes run 1 cyc/row (fp32 takes 2).
  - ATTN_X=bf16 (default): x host-cast to bf16; xbar DMA-transpose loads
    x^T [768, 2048] straight from DRAM (no PE transposes, no psum copies
    for x at all).  ATTN_X=f32 keeps x f32 in DRAM: f32 DMA + engine-split
    cast (Pool/ACT/DVE) + bf16 PE transposes.
  - x^T is double-buffered across reps; the next rep's 24 transpose-DMAs
    are emitted right after each range's output DMA so the SP queue never
    serializes them behind a whole rep's outputs, and x DMA fully overlaps
    the previous rep's attention.

Per-core dataflow:
  1. x^T via DMA-transpose (above).
  2. Projections with concatenated stationary weights [Wq|Wk], [Wv|Qq]:
     [128, 2, 512] f32 psum tiles per range-pair; 1024-wide cast-copies to
     QK_sb / VQ_sb bf16 (rows 0-63 Q^T/V^T, rows 64-127 K^T / Q^T-dup --
     the dup puts Q^T on the same partitions as K^T, which matmul requires
     for the S^T operands).  V^T tiles PE-transposed back to V_aug
     [128, 65] bf16 whose ones column makes the O^T matmul emit softmax
     denominators for free.
  3. Attention, software-pipelined per 512-wide query range: stages are
     either a pair of full k-tiles (2 S^T matmuls -> one [128, 2x512] exp
     on ACT, 1/8 scale fused, bf16 out -> 2 O^T matmuls) or the diagonal
     group (4 packed S^T matmuls -> 3 exps -> gpsimd affine_select zeroes
     the invalid q<k half of each diagonal 128x128 block post-exp -> 4 O^T
     matmuls).  The S^T matmuls of stage t+1 are issued BEFORE the O^T
     matmuls of stage t so the exp latency hides behind PE compute; the
     diagonal stage sits second in each range so pair stages cover its
     longer exp+affine chain.  O^T accumulates over k in a [65, 512] psum.
  4. Per-range normalize/output is deferred one stage into the next range:
     O^T+rowsum -> bf16 copy -> PE transpose back to [128, 4, 66] psum ->
     reciprocal + scale on DVE -> one batched output DMA per range.

Perf notes (measured by rep-differencing on HW): fp32r baseline 67.7 us
per rep; this kernel ~33 us.  PE cost on real HW is approximately
sum(stream columns + stationary rows) per matmul -- stationary loads are
NOT hidden (no ldw-opt in this toolchain), which is why V is computed via
the packed [Wv|Wq] pass (12 loads) rather than x^T-stationary per-tile
matmuls (96 loads), and why fp8 DoubleRow (max 64 psum partitions, kills
the ones-column denominator trick) did not pay off.  gpsimd software-DGE
DMAs are catastrophically slow here (DynamicDMA disabled) -- use SP/ACT
HWDGE queues only.
"""

import os
import numpy as np

import concourse.tile as tile
from concourse import bacc, mybir
from concourse.bass_utils import run_bass_kernel_spmd
from concourse.masks import make_identity

P = 128
N = 2048
D = 768
H = 64
NT = N // P   # 16 n-tiles
DT = D // P   # 6 d-tiles
W = 512       # q-range width
QR = N // W   # 4 q-ranges
F32 = mybir.dt.float32
BF = mybir.dt.bfloat16

# bf16 x-path: host-cast x to bf16, xbar DMA-transpose straight from DRAM.
BF16_X = os.environ.get("ATTN_X", "bf16") == "bf16"

# Three of the four diagonal tiles (widths 512, 384, 128) pack into one
# [128, 1024] psum (bank0: 512 | bank1: 384+128); the 256-wide one (jj=2)
# gets its own narrow tile.  No matmul output crosses a 512-col f32 bank.
DIAG_PACK = {0: (0, 512), 1: (512, 384), 3: (896, 128)}  # jj -> (off, width)
DIAG_TOT = 1024


def build_kernel(reps=1):
    nc = bacc.Bacc(name="attn_head")
    x_d = nc.dram_tensor("x", [N, D], BF if BF16_X else F32,
                         kind="ExternalInput")
    wq_d = nc.dram_tensor("Wq", [D, H], F32, kind="ExternalInput")
    wk_d = nc.dram_tensor("Wk", [D, H], F32, kind="ExternalInput")
    wv_d = nc.dram_tensor("Wv", [D, H], F32, kind="ExternalInput")
    out_d = nc.dram_tensor("out", [N, H], F32, kind="ExternalOutput")

    Exp = mybir.ActivationFunctionType.Exp

    with tile.TileContext(nc) as tc:
        with (
            tc.tile_pool(name="persist", bufs=1) as persist,
            tc.tile_pool(name="xload", bufs=2) as xload,
            tc.tile_pool(name="work", bufs=3) as work,
            tc.tile_pool(name="psum", bufs=1, space="PSUM") as psum,
        ):
            # warm the ACT exp table while DMAs run
            dummy = persist.tile([P, 1], F32)
            nc.vector.memset(dummy[:], 0.0)
            nc.scalar.activation(dummy[:], dummy[:], Exp)

            ident = persist.tile([P, P], F32)
            make_identity(nc, ident[:])
            ident_bf = persist.tile([P, P], BF)
            nc.vector.tensor_copy(out=ident_bf[:], in_=ident[:])

            ones_col = persist.tile([P, 1], F32)
            nc.vector.memset(ones_col[:], 1.0)

            xT = persist.tile([P, 2, DT, N], BF)     # x^T: [d%128, buf, d//128, n]
            QK_sb = persist.tile([P, 2, N], BF)      # rows 0-63 Q^T, 64-127 K^T
            VQ_sb = persist.tile([P, 2, N], BF)      # rows 0-63 V^T, 64-127 Q^T
            Vb = persist.tile([P, 2, NT, H + 1], BF)  # V tiles + ones col
            for s in range(2):
                nc.vector.tensor_copy(
                    out=Vb[:, s, :, H],
                    in_=ones_col[:, 0].to_broadcast((P, NT)),
                )

            # weights as [128, 6, 64]: partition = d%128, tile = d//128.
            # Concatenated pairs [Wq|Wk] and [Wv|Wq] make 128-wide stationary
            # operands: one projection matmul computes two 64-row outputs.
            w_raw = {}
            w_dma = []
            for wname, dram in (("q", wq_d), ("k", wk_d), ("v", wv_d)):
                w_raw[wname] = persist.tile([P, DT, H], F32,
                                            name=f"wraw_{wname}")
                w_dma.append((w_raw[wname], dram))
            w_qk = persist.tile([P, DT, 2 * H], BF)
            w_vq = persist.tile([P, DT, 2 * H], BF)

            def load_weights():
                for w_t, dram in w_dma:
                    nc.sync.dma_start(
                        out=w_t[:],
                        in_=dram[:, :].rearrange("(t p) h -> p t h", p=P),
                    )
                nc.vector.tensor_copy(out=w_qk[:, :, 0:H], in_=w_raw["q"][:])
                nc.vector.tensor_copy(out=w_qk[:, :, H:], in_=w_raw["k"][:])
                nc.vector.tensor_copy(out=w_vq[:, :, 0:H], in_=w_raw["v"][:])
                nc.vector.tensor_copy(out=w_vq[:, :, H:], in_=w_raw["q"][:])

            def emit_xt_dma(rep, groups=range(QR)):
                sl = rep % 2
                for g in groups:
                    ns = slice(g * W, (g + 1) * W)
                    for d_i in range(DT):
                        nc.sync.dma_start(
                            out=xT[:, sl, d_i, ns],
                            in_=x_d[ns, d_i * P:(d_i + 1) * P],
                            transpose=True,
                        )

            for rep in range(reps):
                sl = rep % 2
                # ---- x load + transpose to xT ------------------------------
                if BF16_X:
                    # x arrives bf16: xbar DMA-transpose straight from DRAM.
                    # rep 0 transposes emitted here; later reps' were hoisted
                    # to right after the previous rep's projections so they
                    # never queue behind that rep's output DMAs on SP.
                    if rep == 0:
                        emit_xt_dma(0)
                        load_weights()
                else:
                    # f32 load (SP hwdge), engine-split cast to bf16
                    # (Pool/ACT/DVE share the 16 tiles), bf16 PE transposes
                    xbs = []
                    casts = []
                    for g in range(QR):
                        xf = xload.tile([P, 4, D], F32, tag="xf",
                                        name=f"xf_{rep}_{g}", bufs=2)
                        xb = xload.tile([P, 4, D], BF, tag="xb",
                                        name=f"xb_{rep}_{g}", bufs=2)
                        xbs.append(xb)
                        nc.sync.dma_start(
                            out=xf[:],
                            in_=x_d[g * W:(g + 1) * W, :].rearrange(
                                "(t p) d -> p t d", p=P),
                        )
                        for i in range(4):
                            eng = (nc.gpsimd, nc.gpsimd, nc.scalar,
                                   nc.vector)[i]
                            if eng is nc.scalar:
                                eng.copy(out=xb[:, i, :], in_=xf[:, i, :])
                            else:
                                eng.tensor_copy(out=xb[:, i, :],
                                                in_=xf[:, i, :])
                    if rep == 0:
                        load_weights()
                    for g in range(QR):
                        for d_i in range(DT):
                            pt = psum.tile([P, 4, P], BF, tag="mm", bufs=2)
                            for i in range(4):
                                nc.tensor.transpose(
                                    pt[:, i, :],
                                    xbs[g][:, i, d_i * P:(d_i + 1) * P],
                                    ident_bf[:],
                                )
                            nc.vector.tensor_copy(
                                out=xT[:, sl, d_i, g * W:(g + 1) * W],
                                in_=pt[:])

                # ---- projections (packed weights, range-pair tiles) --------
                for h2 in range(2):  # ranges (2*h2, 2*h2+1)
                    ds = slice(2 * h2 * W, (2 * h2 + 2) * W)
                    pqk2 = psum.tile([P, 2, W], F32, tag="mm", bufs=2,
                                     name=f"pqk2_{rep}_{h2}")
                    pvq2 = psum.tile([P, 2, W], F32, tag="mm", bufs=2,
                                     name=f"pvq2_{rep}_{h2}")
                    for d_i in range(DT):
                        kw = dict(start=(d_i == 0), stop=(d_i == DT - 1),
                                  skip_group_check=True)
                        for u in range(2):
                            rhs = xT[:, sl, d_i, (2 * h2 + u) * W:
                                     (2 * h2 + u + 1) * W]
                            nc.tensor.matmul(pqk2[:, u, :], w_qk[:, d_i],
                                             rhs, **kw)
                    nc.vector.tensor_copy(out=QK_sb[:, sl, ds], in_=pqk2[:])
                    for d_i in range(DT):
                        kw = dict(start=(d_i == 0), stop=(d_i == DT - 1),
                                  skip_group_check=True)
                        for u in range(2):
                            rhs = xT[:, sl, d_i, (2 * h2 + u) * W:
                                     (2 * h2 + u + 1) * W]
                            nc.tensor.matmul(pvq2[:, u, :], w_vq[:, d_i],
                                             rhs, **kw)
                    nc.vector.tensor_copy(out=VQ_sb[:, sl, ds], in_=pvq2[:])
                    # V^T tiles -> V_aug [128, 65] per k-tile (batched copy)
                    pvt = psum.tile([P, 8, H], BF, tag="proj", bufs=2,
                                    name=f"pvt_{rep}_{h2}")
                    for i in range(8):
                        nc.tensor.transpose(
                            pvt[:, i, :],
                            VQ_sb[0:H, sl,
                                  2 * h2 * W + i * P:2 * h2 * W + (i + 1) * P],
                            ident_bf[:H, :H],
                        )
                    nc.vector.tensor_copy(
                        out=Vb[:, sl, 8 * h2:8 * h2 + 8, 0:H], in_=pvt[:])

                # ---- attention: software-pipelined stages ------------------
                KTd = QK_sb[H:P, sl, :]   # K^T on partitions 64-127
                QTd = VQ_sb[H:P, sl, :]   # Q^T duplicate on partitions 64-127
                po_tiles = {}

                def po_for(r):
                    if r not in po_tiles:
                        po_tiles[r] = psum.tile([H + 1, W], F32, tag="po",
                                                bufs=2, name=f"po_{rep}_{r}")
                    return po_tiles[r]

                def make_pair(r, jp):
                    ps2 = {}

                    def emit_S():
                        ps2["t"] = psum.tile([P, 2, W], F32, tag="mm",
                                             bufs=2,
                                             name=f"ps2_{rep}_{r}_{jp}")
                        ps2["e"] = work.tile([P, 2, W], BF, tag="es2",
                                             bufs=6,
                                             name=f"es2_{rep}_{r}_{jp}")
                        qs = slice(r * W, (r + 1) * W)
                        for u in range(2):
                            j = 2 * jp + u
                            nc.tensor.matmul(
                                ps2["t"][:, u, :],
                                KTd[:, j * P:(j + 1) * P],
                                QTd[:, qs], start=True, stop=True,
                            )
                        nc.scalar.activation(ps2["e"][:], ps2["t"][:], Exp,
                                             scale=0.125)

                    def emit_O(first, last):
                        po = po_for(r)
                        for u in range(2):
                            j = 2 * jp + u
                            nc.tensor.matmul(
                                po[:], Vb[:, sl, j, :], ps2["e"][:, u, :],
                                start=(first and u == 0),
                                stop=(last and u == 1),
                                skip_group_check=True,
                            )

                    return emit_S, emit_O

                def make_diag(r):
                    st = {}

                    def emit_S():
                        st["psd"] = psum.tile([P, DIAG_TOT], F32, tag="mm",
                                              bufs=2, name=f"psd_{rep}_{r}")
                        st["esd"] = work.tile([P, DIAG_TOT], BF, tag="esd",
                                              bufs=4, name=f"esd_{rep}_{r}")
                        st["ps1"] = psum.tile([P, 256], F32, tag="proj",
                                              bufs=2, name=f"ps1_{rep}_{r}")
                        st["es1"] = work.tile([P, 256], BF, tag="es1",
                                              bufs=4, name=f"es1_{rep}_{r}")
                        psd, esd = st["psd"], st["esd"]
                        ps1, es1 = st["ps1"], st["es1"]
                        for jj in range(4):
                            j = 4 * r + jj
                            if jj == 2:
                                sv = ps1[:, :]
                            else:
                                poff, wd = DIAG_PACK[jj]
                                sv = psd[:, poff:poff + wd]
                            nc.tensor.matmul(
                                sv,
                                KTd[:, j * P:(j + 1) * P],
                                QTd[:, r * W + jj * P:(r + 1) * W],
                                start=True, stop=True,
                            )
                        nc.scalar.activation(esd[:, 0:W], psd[:, 0:W], Exp,
                                             scale=0.125)
                        nc.scalar.activation(esd[:, W:], psd[:, W:], Exp,
                                             scale=0.125)
                        nc.scalar.activation(es1[:], ps1[:], Exp, scale=0.125)
                        # zero the invalid (q < k) half of each diagonal
                        # 128x128 block post-exp on the idle GPSIMD engine
                        for jj in range(4):
                            ev = es1[:, 0:P] if jj == 2 else (
                                esd[:, DIAG_PACK[jj][0]:
                                    DIAG_PACK[jj][0] + P])
                            nc.gpsimd.affine_select(
                                out=ev, in_=ev,
                                compare_op=mybir.AluOpType.is_ge,
                                fill=0.0, base=0,
                                pattern=[[1, P]], channel_multiplier=-1,
                            )

                    def emit_O(first, last):
                        po = po_for(r)
                        esd, es1 = st["esd"], st["es1"]
                        for jj in range(4):
                            j = 4 * r + jj
                            if jj == 2:
                                rhs = es1[:, :]
                            else:
                                poff, wd = DIAG_PACK[jj]
                                rhs = esd[:, poff:poff + wd]
                            nc.tensor.matmul(
                                po[:, jj * P:],
                                Vb[:, sl, j, :],
                                rhs,
                                start=(first and jj == 0),
                                stop=(last and jj == 3),
                                skip_group_check=True,
                            )

                    return emit_S, emit_O

                def final_dve(r):
                    ot = work.tile([H + 1, W], BF, tag="ot", bufs=4,
                                   name=f"ot_{rep}_{r}")
                    nc.vector.tensor_copy(out=ot[:], in_=po_tiles[r][:])
                    return ot

                def final_pe(r, ot):
                    pf = psum.tile([P, 4, H + 2], BF, tag="proj", bufs=2,
                                   name=f"pf_{rep}_{r}")
                    ob = work.tile([P, 4, H], F32, tag="ob",
                                   name=f"ob_{rep}_{r}", bufs=3)
                    for i in range(4):
                        nc.tensor.transpose(
                            pf[:, i, 0:H + 1], ot[:, i * P:(i + 1) * P],
                            ident_bf[:H + 1, :H + 1],
                        )
                        rs = work.tile([P, 1], F32, tag="rs",
                                       name=f"rs_{rep}_{r}_{i}")
                        nc.vector.reciprocal(rs[:], pf[:, i, H:H + 1])
                        nc.vector.tensor_scalar_mul(
                            ob[:, i, :], pf[:, i, 0:H], rs[:]
                        )
                    nc.sync.dma_start(
                        out=out_d[r * W:(r + 1) * W, :].rearrange(
                            "(t p) h -> p t h", p=P),
                        in_=ob[:],
                    )
                    if BF16_X and rep + 1 < reps:
                        emit_xt_dma(rep + 1, groups=[r])

                # build stage list: diag scheduled second in each range so
                # pair stages cover its exp+affine latency (r=0: alone)
                stages = []  # (emit_S, emit_O, r, first, last)
                for r in range(QR):
                    pairs = [make_pair(r, jp) for jp in range(2 * r)]
                    dstage = make_diag(r)
                    if pairs:
                        order = [pairs[0], dstage] + pairs[1:]
                    else:
                        order = [dstage]
                    for i, (eS, eO) in enumerate(order):
                        stages.append((eS, eO, r, i == 0,
                                       i == len(order) - 1))

                stages[0][0]()          # S of stage 0
                if len(stages) > 1:
                    stages[1][0]()      # S of stage 1 (lookahead 2)
                pending = None          # (r, ot) awaiting final_pe
                for t, (eS, eO, r, first, last) in enumerate(stages):
                    if t + 2 < len(stages):
                        stages[t + 2][0]()   # S two stages ahead
                    eO(first, last)
                    if pending is not None:
                        final_pe(*pending)
                        pending = None
                    if last:
                        pending = (r, final_dve(r))
                if pending is not None:
                    final_pe(*pending)

    nc.compile()
    return nc


_NC_CACHE = {}


def _get_nc(reps=1):
    if reps not in _NC_CACHE:
        _NC_CACHE[reps] = build_kernel(reps)
    return _NC_CACHE[reps]


def kernel(input, Wq, Wk, Wv, **_unused):
    if BF16_X:
        import ml_dtypes
        input = np.ascontiguousarray(
            np.asarray(input).astype(ml_dtypes.bfloat16))
    else:
        input = np.ascontiguousarray(np.asarray(input, dtype=np.float32))
    Wq = np.ascontiguousarray(np.asarray(Wq, dtype=np.float32))
    Wk = np.ascontiguousarray(np.asarray(Wk, dtype=np.float32))
    Wv = np.ascontiguousarray(np.asarray(Wv, dtype=np.float32))
    B = input.shape[0]
    assert B == 8 and input.shape[1] == N and input.shape[2] == D

    nc = _get_nc()
    in_maps = [
        {"x": input[b], "Wq": Wq, "Wk": Wk, "Wv": Wv} for b in range(B)
    ]
    res = run_bass_kernel_spmd(nc, in_maps, core_ids=list(range(B)))
    return np.stack([res.results[b]["out"] for b in range(B)], axis=0)


# revision 18
# speedup vs baseline: 1.4608x; 1.4608x over previous
"""Bass/Tile kernel for a single causal attention head on 8 trn2 NeuronCores.

Problem: input [8, 2048, 768], Wq/Wk/Wv [768, 64] ->
  O = softmax(causal(Q K^T)/sqrt(64)) V, per batch.  [8, 2048, 64]

Sharding: data-parallel over batch; core b handles batch b. Weights
replicated.  No collectives needed.

All-bf16 matmul pipeline (tolerance 2e-2; measured rel err ~5e-3):
  - bf16 matmuls run 1 cyc/row incl. <256-wide outputs (fp32r degrades to
    4 there), bf16 PE transposes run 1 cyc/row (fp32 takes 2).
  - ATTN_X=bf16 (default): x host-cast to bf16; xbar DMA-transpose loads
    x^T [768, 2048] straight from DRAM (no PE transposes, no psum copies
    for x at all).  ATTN_X=f32 keeps x f32 in DRAM: f32 DMA + engine-split
    cast (Pool/ACT/DVE) + bf16 PE transposes.
  - x^T is double-buffered across reps; the next rep's 24 transpose-DMAs
    are emitted right after each range's output DMA so the SP queue never
    serializes them behind a whole rep's outputs, and x DMA fully overlaps
    the previous rep's attention.

Per-core dataflow:
  1. x^T via DMA-transpose (above).
  2. Projections with concatenated stationary weights [Wq|Wk], [Wv|Qq]:
     [128, 2, 512] f32 psum tiles per range-pair; 1024-wide cast-copies to
     QK_sb / VQ_sb bf16 (rows 0-63 Q^T/V^T, rows 64-127 K^T / Q^T-dup --
     the dup puts Q^T on the same partitions as K^T, which matmul requires
     for the S^T operands).  V^T tiles PE-transposed back to V_aug
     [128, 65] bf16 whose ones column makes the O^T matmul emit softmax
     denominators for free.
  3. Attention, software-pipelined per 512-wide query range: stages are
     either a pair of full k-tiles (2 S^T matmuls -> one [128, 2x512] exp
     on ACT, 1/8 scale fused, bf16 out -> 2 O^T matmuls) or the diagonal
     group (4 packed S^T matmuls -> 3 exps -> gpsimd affine_select zeroes
     the invalid q<k half of each diagonal 128x128 block post-exp -> 4 O^T
     matmuls).  The S^T matmuls of stage t+1 are issued BEFORE the O^T
     matmuls of stage t so the exp latency hides behind PE compute; the
     diagonal stage sits second in each range so pair stages cover its
     longer exp+affine chain.  O^T accumulates over k in a [65, 512] psum.
  4. Per-range normalize/output is deferred one stage into the next range:
     O^T+rowsum -> bf16 copy -> PE transpose back to [128, 4, 66] psum ->
     reciprocal + scale on DVE -> one batched output DMA per range.

Perf notes (measured by rep-differencing on HW): fp32r baseline 67.7 us
per rep; this kernel ~33 us.  PE cost on real HW is approximately
sum(stream columns + stationary rows) per matmul -- stationary loads are
NOT hidden (no ldw-opt in this toolchain), which is why V is computed via
the packed [Wv|Wq] pass (12 loads) rather than x^T-stationary per-tile
matmuls (96 loads), and why fp8 DoubleRow (max 64 psum partitions, kills
the ones-column denominator trick) did not pay off.  gpsimd software-DGE
DMAs are catastrophically slow here (DynamicDMA disabled) -- use SP/ACT
HWDGE queues only.
"""

import os
import numpy as np

import concourse.tile as tile
from concourse import bacc, mybir
from concourse.bass_utils import run_bass_kernel_spmd
from concourse.masks import make_identity

P = 128
N = 2048
D = 768
H = 64
NT = N // P   # 16 n-tiles
DT = D // P   # 6 d-tiles
W = 512       # q-range width
QR = N // W   # 4 q-ranges
F32 = mybir.dt.float32
BF = mybir.dt.bfloat16

# bf16 x-path: host-cast x to bf16, xbar DMA-transpose straight from DRAM.
BF16_X = os.environ.get("ATTN_X", "bf16") == "bf16"

# Three of the four diagonal tiles (widths 512, 384, 128) pack into one
# [128, 1024] psum (bank0: 512 | bank1: 384+128); the 256-wide one (jj=2)
# gets its own narrow tile.  No matmul output crosses a 512-col f32 bank.
DIAG_PACK = {0: (0, 512), 1: (512, 384), 3: (896, 128)}  # jj -> (off, width)
DIAG_TOT = 1024


def build_kernel(reps=1):
    nc = bacc.Bacc(name="attn_head")
    x_d = nc.dram_tensor("x", [N, D], BF if BF16_X else F32,
                         kind="ExternalInput")
    wq_d = nc.dram_tensor("Wq", [D, H], F32, kind="ExternalInput")
    wk_d = nc.dram_tensor("Wk", [D, H], F32, kind="ExternalInput")
    wv_d = nc.dram_tensor("Wv", [D, H], F32, kind="ExternalInput")
    out_d = nc.dram_tensor("out", [N, H], F32, kind="ExternalOutput")

    Exp = mybir.ActivationFunctionType.Exp

    with tile.TileContext(nc) as tc:
        with (
            tc.tile_pool(name="persist", bufs=1) as persist,
            tc.tile_pool(name="xload", bufs=2) as xload,
            tc.tile_pool(name="work", bufs=3) as work,
            tc.tile_pool(name="psum", bufs=1, space="PSUM") as psum,
        ):
            # warm the ACT exp table while DMAs run
            dummy = persist.tile([P, 1], F32)
            nc.vector.memset(dummy[:], 0.0)
            nc.scalar.activation(dummy[:], dummy[:], Exp)

            ident = persist.tile([P, P], F32)
            make_identity(nc, ident[:])
            ident_bf = persist.tile([P, P], BF)
            nc.vector.tensor_copy(out=ident_bf[:], in_=ident[:])

            ones_col = persist.tile([P, 1], F32)
            nc.vector.memset(ones_col[:], 1.0)

            xT = persist.tile([P, 2, DT, N], BF)     # x^T: [d%128, buf, d//128, n]
            QK_sb = persist.tile([P, 2, N], BF)      # rows 0-63 Q^T, 64-127 K^T
            VQ_sb = persist.tile([P, 2, N], BF)      # rows 0-63 V^T, 64-127 Q^T
            Vb = persist.tile([P, 2, NT, H + 1], BF)  # V tiles + ones col
            for s in range(2):
                nc.vector.tensor_copy(
                    out=Vb[:, s, :, H],
                    in_=ones_col[:, 0].to_broadcast((P, NT)),
                )

            # weights as [128, 6, 64]: partition = d%128, tile = d//128.
            # Concatenated pairs [Wq|Wk] and [Wv|Wq] make 128-wide stationary
            # operands: one projection matmul computes two 64-row outputs.
            w_raw = {}
            w_dma = []
            for wname, dram in (("q", wq_d), ("k", wk_d), ("v", wv_d)):
                w_raw[wname] = persist.tile([P, DT, H], F32,
                                            name=f"wraw_{wname}")
                w_dma.append((w_raw[wname], dram))
            w_qk = persist.tile([P, DT, 2 * H], BF)
            w_vq = persist.tile([P, DT, 2 * H], BF)

            def load_weights():
                for w_t, dram in w_dma:
                    nc.sync.dma_start(
                        out=w_t[:],
                        in_=dram[:, :].rearrange("(t p) h -> p t h", p=P),
                    )
                nc.vector.tensor_copy(out=w_qk[:, :, 0:H], in_=w_raw["q"][:])
                nc.vector.tensor_copy(out=w_qk[:, :, H:], in_=w_raw["k"][:])
                nc.vector.tensor_copy(out=w_vq[:, :, 0:H], in_=w_raw["v"][:])
                nc.vector.tensor_copy(out=w_vq[:, :, H:], in_=w_raw["q"][:])

            def emit_xt_dma(rep, groups=range(QR)):
                sl = rep % 2
                for g in groups:
                    ns = slice(g * W, (g + 1) * W)
                    for d_i in range(DT):
                        nc.sync.dma_start(
                            out=xT[:, sl, d_i, ns],
                            in_=x_d[ns, d_i * P:(d_i + 1) * P],
                            transpose=True,
                        )

            for rep in range(reps):
                sl = rep % 2
                # ---- x load + transpose to xT ------------------------------
                if BF16_X:
                    # x arrives bf16: xbar DMA-transpose straight from DRAM.
                    # rep 0 transposes emitted here; later reps' were hoisted
                    # to right after the previous rep's projections so they
                    # never queue behind that rep's output DMAs on SP.
                    if rep == 0:
                        emit_xt_dma(0)
                        load_weights()
                else:
                    # f32 load (SP hwdge), engine-split cast to bf16
                    # (Pool/ACT/DVE share the 16 tiles), bf16 PE transposes
                    xbs = []
                    casts = []
                    for g in range(QR):
                        xf = xload.tile([P, 4, D], F32, tag="xf",
                                        name=f"xf_{rep}_{g}", bufs=2)
                        xb = xload.tile([P, 4, D], BF, tag="xb",
                                        name=f"xb_{rep}_{g}", bufs=2)
                        xbs.append(xb)
                        nc.sync.dma_start(
                            out=xf[:],
                            in_=x_d[g * W:(g + 1) * W, :].rearrange(
                                "(t p) d -> p t d", p=P),
                        )
                        for i in range(4):
                            eng = (nc.gpsimd, nc.gpsimd, nc.scalar,
                                   nc.vector)[i]
                            if eng is nc.scalar:
                                eng.copy(out=xb[:, i, :], in_=xf[:, i, :])
                            else:
                                eng.tensor_copy(out=xb[:, i, :],
                                                in_=xf[:, i, :])
                    if rep == 0:
                        load_weights()
                    for g in range(QR):
                        for d_i in range(DT):
                            pt = psum.tile([P, 4, P], BF, tag="mm", bufs=2)
                            for i in range(4):
                                nc.tensor.transpose(
                                    pt[:, i, :],
                                    xbs[g][:, i, d_i * P:(d_i + 1) * P],
                                    ident_bf[:],
                                )
                            nc.vector.tensor_copy(
                                out=xT[:, sl, d_i, g * W:(g + 1) * W],
                                in_=pt[:])

                # ---- projections (packed weights, range-pair tiles) --------
                for h2 in range(2):  # ranges (2*h2, 2*h2+1)
                    ds = slice(2 * h2 * W, (2 * h2 + 2) * W)
                    pqk2 = psum.tile([P, 2, W], F32, tag="mm", bufs=2,
                                     name=f"pqk2_{rep}_{h2}")
                    pvq2 = psum.tile([P, 2, W], F32, tag="mm", bufs=2,
                                     name=f"pvq2_{rep}_{h2}")
                    for d_i in range(DT):
                        kw = dict(start=(d_i == 0), stop=(d_i == DT - 1),
                                  skip_group_check=True)
                        for u in range(2):
                            rhs = xT[:, sl, d_i, (2 * h2 + u) * W:
                                     (2 * h2 + u + 1) * W]
                            nc.tensor.matmul(pqk2[:, u, :], w_qk[:, d_i],
                                             rhs, **kw)
                    nc.vector.tensor_copy(out=QK_sb[:, sl, ds], in_=pqk2[:])
                    for d_i in range(DT):
                        kw = dict(start=(d_i == 0), stop=(d_i == DT - 1),
                                  skip_group_check=True)
                        for u in range(2):
                            rhs = xT[:, sl, d_i, (2 * h2 + u) * W:
                                     (2 * h2 + u + 1) * W]
                            nc.tensor.matmul(pvq2[:, u, :], w_vq[:, d_i],
                                             rhs, **kw)
                    nc.vector.tensor_copy(out=VQ_sb[:, sl, ds], in_=pvq2[:])
                    # V^T tiles -> V_aug [128, 65] per k-tile (batched copy)
                    pvt = psum.tile([P, 8, H], BF, tag="proj", bufs=2,
                                    name=f"pvt_{rep}_{h2}")
                    for i in range(8):
                        nc.tensor.transpose(
                            pvt[:, i, :],
                            VQ_sb[0:H, sl,
                                  2 * h2 * W + i * P:2 * h2 * W + (i + 1) * P],
                            ident_bf[:H, :H],
                        )
                    nc.vector.tensor_copy(
                        out=Vb[:, sl, 8 * h2:8 * h2 + 8, 0:H], in_=pvt[:])

                # ---- attention: software-pipelined stages ------------------
                KTd = QK_sb[H:P, sl, :]   # K^T on partitions 64-127
                QTd = VQ_sb[H:P, sl, :]   # Q^T duplicate on partitions 64-127
                po_tiles = {}

                def po_for(r):
                    if r not in po_tiles:
                        po_tiles[r] = psum.tile([H + 1, W], F32, tag="po",
                                                bufs=2, name=f"po_{rep}_{r}")
                    return po_tiles[r]

                def make_pair(r, jp):
                    ps2 = {}

                    def emit_S():
                        ps2["t"] = psum.tile([P, 2, W], F32, tag="mm",
                                             bufs=2,
                                             name=f"ps2_{rep}_{r}_{jp}")
                        ps2["e"] = work.tile([P, 2, W], BF, tag="es2",
                                             bufs=6,
                                             name=f"es2_{rep}_{r}_{jp}")
                        qs = slice(r * W, (r + 1) * W)
                        for u in range(2):
                            j = 2 * jp + u
                            nc.tensor.matmul(
                                ps2["t"][:, u, :],
                                KTd[:, j * P:(j + 1) * P],
                                QTd[:, qs], start=True, stop=True,
                            )
                        nc.scalar.activation(ps2["e"][:], ps2["t"][:], Exp,
                                             scale=0.125)

                    def emit_O(first, last):
                        po = po_for(r)
                        for u in range(2):
                            j = 2 * jp + u
                            nc.tensor.matmul(
                                po[:], Vb[:, sl, j, :], ps2["e"][:, u, :],
                                start=(first and u == 0),
                                stop=(last and u == 1),
                                skip_group_check=True,
                            )

                    return emit_S, emit_O

                def make_diag(r):
                    st = {}

                    def emit_S():
                        st["psd"] = psum.tile([P, DIAG_TOT], F32, tag="mm",
                                              bufs=2, name=f"psd_{rep}_{r}")
                        st["esd"] = work.tile([P, DIAG_TOT], BF, tag="esd",
                                              bufs=4, name=f"esd_{rep}_{r}")
                        st["ps1"] = psum.tile([P, 256], F32, tag="proj",
                                              bufs=2, name=f"ps1_{rep}_{r}")
                        st["es1"] = work.tile([P, 256], BF, tag="es1",
                                              bufs=4, name=f"es1_{rep}_{r}")
                        psd, esd = st["psd"], st["esd"]
                        ps1, es1 = st["ps1"], st["es1"]
                        for jj in range(4):
                            j = 4 * r + jj
                            if jj == 2:
                                sv = ps1[:, :]
                            else:
                                poff, wd = DIAG_PACK[jj]
                                sv = psd[:, poff:poff + wd]
                            nc.tensor.matmul(
                                sv,
                                KTd[:, j * P:(j + 1) * P],
                                QTd[:, r * W + jj * P:(r + 1) * W],
                                start=True, stop=True,
                            )
                        nc.scalar.activation(esd[:, 0:W], psd[:, 0:W], Exp,
                                             scale=0.125)
                        nc.scalar.activation(esd[:, W:], psd[:, W:], Exp,
                                             scale=0.125)
                        nc.scalar.activation(es1[:], ps1[:], Exp, scale=0.125)
                        # zero the invalid (q < k) half of each diagonal
                        # 128x128 block post-exp on the idle GPSIMD engine
                        for jj in range(4):
                            ev = es1[:, 0:P] if jj == 2 else (
                                esd[:, DIAG_PACK[jj][0]:
                                    DIAG_PACK[jj][0] + P])
                            nc.gpsimd.affine_select(
                                out=ev, in_=ev,
                                compare_op=mybir.AluOpType.is_ge,
                                fill=0.0, base=0,
                                pattern=[[1, P]], channel_multiplier=-1,
                            )

                    def emit_O(first, last):
                        po = po_for(r)
                        esd, es1 = st["esd"], st["es1"]
                        for jj in range(4):
                            j = 4 * r + jj
                            if jj == 2:
                                rhs = es1[:, :]
                            else:
                                poff, wd = DIAG_PACK[jj]
                                rhs = esd[:, poff:poff + wd]
                            nc.tensor.matmul(
                                po[:, jj * P:],
                                Vb[:, sl, j, :],
                                rhs,
                                start=(first and jj == 0),
                                stop=(last and jj == 3),
                                skip_group_check=True,
                            )

                    return emit_S, emit_O

                def final_dve(r):
                    ot = work.tile([H + 1, W], BF, tag="ot", bufs=4,
                                   name=f"ot_{rep}_{r}")
                    nc.vector.tensor_copy(out=ot[:], in_=po_tiles[r][:])
                    return ot

                def final_pe(r, ot):
                    pf = psum.tile([P, 4, H + 2], BF, tag="proj", bufs=2,
                                   name=f"pf_{rep}_{r}")
                    ob = work.tile([P, 4, H], F32, tag="ob",
                                   name=f"ob_{rep}_{r}", bufs=3)
                    for i in range(4):
                        nc.tensor.transpose(
                            pf[:, i, 0:H + 1], ot[:, i * P:(i + 1) * P],
                            ident_bf[:H + 1, :H + 1],
                        )
                        rs = work.tile([P, 1], F32, tag="rs",
                                       name=f"rs_{rep}_{r}_{i}")
                        nc.vector.reciprocal(rs[:], pf[:, i, H:H + 1])
                        nc.vector.tensor_scalar_mul(
                            ob[:, i, :], pf[:, i, 0:H], rs[:]
                        )
                    nc.sync.dma_start(
                        out=out_d[r * W:(r + 1) * W, :].rearrange(
                            "(t p) h -> p t h", p=P),
                        in_=ob[:],
                    )
                    if BF16_X and rep + 1 < reps:
                        emit_xt_dma(rep + 1, groups=[r])

                # build stage list: diag scheduled second in each range so
                # pair stages cover its exp+affine latency (r=0: alone)
                stages = []  # (emit_S, emit_O, r, first, last)
                for r in range(QR):
                    pairs = [make_pair(r, jp) for jp in range(2 * r)]
                    dstage = make_diag(r)
                    if pairs:
                        order = [pairs[0], dstage] + pairs[1:]
                    else:
                        order = [dstage]
                    for i, (eS, eO) in enumerate(order):
                        stages.append((eS, eO, r, i == 0,
                                       i == len(order) - 1))

                stages[0][0]()          # S of stage 0
                pending = None          # (r, ot) awaiting final_pe
                for t, (eS, eO, r, first, last) in enumerate(stages):
                    if t + 1 < len(stages):
                        stages[t + 1][0]()   # S of next stage
                    eO(first, last)
                    if pending is not None:
                        final_pe(*pending)
                        pending = None
                    if last:
                        pending = (r, final_dve(r))
                if pending is not None:
                    final_pe(*pending)

    nc.compile()
    return nc


_NC_CACHE = {}


def _get_nc(reps=1):
    if reps not in _NC_CACHE:
        _NC_CACHE[reps] = build_kernel(reps)
    return _NC_CACHE[reps]


def kernel(input, Wq, Wk, Wv, **_unused):
    if BF16_X:
        import ml_dtypes
        input = np.ascontiguousarray(
            np.asarray(input).astype(ml_dtypes.bfloat16))
    else:
        input = np.ascontiguousarray(np.asarray(input, dtype=np.float32))
    Wq = np.ascontiguousarray(np.asarray(Wq, dtype=np.float32))
    Wk = np.ascontiguousarray(np.asarray(Wk, dtype=np.float32))
    Wv = np.ascontiguousarray(np.asarray(Wv, dtype=np.float32))
    B = input.shape[0]
    assert B == 8 and input.shape[1] == N and input.shape[2] == D

    nc = _get_nc()
    in_maps = [
        {"x": input[b], "Wq": Wq, "Wk": Wk, "Wv": Wv} for b in range(B)
    ]
    res = run_bass_kernel_spmd(nc, in_maps, core_ids=list(range(B)))
    return np.stack([res.results[b]["out"] for b in range(B)], axis=0)
